# revision 10
# baseline (speedup 1.0000x reference)
"""LAHGCN hypergraph-conv kernel for 8 Trainium2 NeuronCores (bf16).

Math (per reference):
  smooth(x) = Dv^-1/2 H De^-1 H^T Dv^-1/2 x  (S),  branches k=0..3:
  hidden_k = relu(S(x_k W1_k + 1 b1_k));  out = concat(hidden) W2 + b2;  res = S out.

Strategy: nodes sharded 8-way, edges sharded 8-way, AllGather between sides.
First smooth (C=1024): indirect dma_gather of rows + one-hot matmul on
TensorE with statically-baked index/segment streams (bf16 tables).
Second smooth (C=40, padded 128): hybrid - a third of the blocks stream a
host-built dense-H block slab (bf16 counts) through plain TensorE matmuls
(zero descriptor-generation), the rest use dma_gather + one-hot (GpSimd);
the two routes run on different engines concurrently.
Degree scalings folded into y (dv), the edge passes (de) and post-W2 (dv^2);
b1 via rank-1 matmul; b2 via host-side rank-1 s1 = S@1 correction.
"""
import numpy as np
import ml_dtypes

BF16 = ml_dtypes.bfloat16
N, E, NNZ = 50000, 20000, 1600000
CONCAT, C_IN, C_HID = 4, 256, 256
C = CONCAT * C_HID            # 1024
C_OUT, C_OUT_P = 40, 128
W = 8
NPC_R, EPC_R = N // W, E // W           # 6250, 2500 real per core
NBLK, EBLK = 49, 20
NPC, EPC = NBLK * 128, EBLK * 128       # 6272, 2560 padded per core
NP_, EP_ = W * NPC, W * EPC             # 50176, 20480
NCH, ECH = NP_ // 128, EP_ // 128       # 392, 160 global 128-chunks
NCG, ECG = NCH // 8, ECH // 8           # 49, 20 groups of 8 chunks
DSLAB, ESLAB = (EBLK + 2) // 3, (NBLK + 2) // 3   # 7, 17 slab blocks
NHALF = NP_ // 2                        # 25088
BATCH = 8                               # gather chunks per dma_gather (1024
                                        # idx = the max one gather supports)


def _wrap_idx(idx):
    """[L] int -> [128, L/16] int16 wrapped layout, replicated across q7 cores."""
    L = len(idx)
    assert L % 16 == 0
    a = np.full((16, L // 16), 0, np.int16)
    a[np.arange(L) % 16, np.arange(L) // 16] = idx.astype(np.int16)
    return np.tile(a, (8, 1))


def _streams_var(rows, segpos, kbs):
    """Flat index stream + seg table with per-block chunk counts kbs."""
    total = sum(kbs)
    idx = np.zeros(total * 128, np.int64)
    seg = np.full((128, total), -1.0, np.float32)
    off = 0
    for r, p, kb in zip(rows, segpos, kbs):
        n = len(r)
        assert n <= kb * 128
        idx[off * 128:off * 128 + n] = r
        cols = off + np.arange(n) // 128
        seg[np.arange(n) % 128, cols] = p.astype(np.float32)
        off += kb
    return idx, seg.astype(BF16)


def _prep(node_idx, edge_idx, dv_is, de_inv):
    """Host-side prep: gather streams for the first smooth + odd blocks of
    the second, dense block slabs for the second smooth's slab blocks."""
    nrow = (node_idx // NPC_R) * NPC + node_idx % NPC_R    # node -> y row
    erow = (edge_idx // EPC_R) * EPC + edge_idx % EPC_R    # edge -> ef row
    p1 = np.argsort(edge_idx, kind="stable")
    e1, n1 = edge_idx[p1], nrow[p1]
    p2 = np.argsort(node_idx, kind="stable")
    n2, e2 = node_idx[p2], erow[p2]
    per = []
    for c in range(W):
        m1 = (e1 >= c * EPC_R) & (e1 < (c + 1) * EPC_R)
        el = e1[m1] - c * EPC_R
        nr = n1[m1]
        lo_rows, lo_pos, hi_rows, hi_pos = [], [], [], []
        for b in range(EBLK):
            mb = (el >= b * 128) & (el < (b + 1) * 128)
            rb, pb = nr[mb], el[mb] - b * 128
            lo = rb < NHALF
            lo_rows.append(rb[lo]); lo_pos.append(pb[lo])
            hi_rows.append(rb[~lo] - NHALF); hi_pos.append(pb[~lo])
        m2 = (n2 >= c * NPC_R) & (n2 < (c + 1) * NPC_R)
        nl = n2[m2] - c * NPC_R
        er = e2[m2]
        c_rows, c_pos = [], []
        for b in range(NBLK):
            mb = (nl >= b * 128) & (nl < (b + 1) * 128)
            c_rows.append(er[mb]); c_pos.append(nl[mb] - b * 128)
        per.append((lo_rows, lo_pos, hi_rows, hi_pos, c_rows, c_pos,
                    el, nr, nl, er))
    kbA = [max(1, max((len(p[0][b]) + 127) // 128 for p in per)) for b in range(EBLK)]
    kbB = [max(1, max((len(p[2][b]) + 127) // 128 for p in per)) for b in range(EBLK)]
    kbC = [max(1, max((len(p[4][b]) + 127) // 128 for p in per)) for b in range(NBLK)]
    cores = []
    for c in range(W):
        lo_rows, lo_pos, hi_rows, hi_pos, c_rows, c_pos, el, nr, nl, er = per[c]
        iA, sA = _streams_var(lo_rows, lo_pos, kbA)
        iB, sB = _streams_var(hi_rows, hi_pos, kbB)
        iC, sC = _streams_var(c_rows, c_pos, kbC)
        dv = np.zeros(NPC, np.float32)
        dv[:NPC_R] = dv_is[c * NPC_R:(c + 1) * NPC_R]
        de = np.zeros(EPC, np.float32)
        de[:EPC_R] = de_inv[c * EPC_R:(c + 1) * EPC_R]
        # dense slabD[n_pad, e_loc] counts, b%3==0 edge blocks only
        sD = np.zeros((NP_, EPC), np.float32)
        np.add.at(sD, (nr, el), 1.0)
        sD = sD.reshape(NCH, 128, EBLK, 128).transpose(2, 0, 1, 3)[0::3]
        sD = sD.reshape(DSLAB, NCG, 8, 128, 128).transpose(0, 1, 3, 2, 4)
        slabD = sD.astype(BF16)
        del sD
        # dense slabE[e_pad, n_loc] counts, b%3==0 node blocks only
        sE = np.zeros((EP_, NPC), np.float32)
        np.add.at(sE, (er, nl), 1.0)
        sE = sE.reshape(ECH, 128, NBLK, 128).transpose(2, 0, 1, 3)[0::3]
        sE = sE.reshape(ESLAB, ECG, 8, 128, 128).transpose(0, 1, 3, 2, 4)
        slabE = sE.astype(BF16)
        del sE
        cores.append(dict(
            idxA=_wrap_idx(iA), segA=sA, idxB=_wrap_idx(iB), segB=sB,
            idxC=_wrap_idx(iC), segC=sC, slabD=slabD, slabE=slabE,
            dv_blk=dv.reshape(NBLK, 128).T.copy(),
            dvsq_blk=(dv * dv).reshape(NBLK, 128).T.copy(),
            de_blk=de.reshape(EBLK, 128).T.copy()))
    return cores, tuple(kbA), tuple(kbB), tuple(kbC)


def _build(kbA, kbB, kbC):
    import concourse.bass as bass
    import concourse.mybir as mybir
    from concourse import bacc, masks
    from concourse.tile import TileContext

    f32, bf16, i16 = mybir.dt.float32, mybir.dt.bfloat16, mybir.dt.int16
    sumA, sumB, sumC = sum(kbA), sum(kbB), sum(kbC)
    oA = np.concatenate([[0], np.cumsum(kbA)]).tolist()
    oB = np.concatenate([[0], np.cumsum(kbB)]).tolist()
    oC = np.concatenate([[0], np.cumsum(kbC)]).tolist()
    # packed offsets for the odd-block re-loads in phases D/E
    oddE = [b for b in range(EBLK) if b % 3 != 0]
    oddN = [b for b in range(NBLK) if b % 3 != 0]
    sumA2 = sum(kbA[b] for b in oddE); sumB2 = sum(kbB[b] for b in oddE)
    sumC2 = sum(kbC[b] for b in oddN)
    oA2, oB2, oC2 = {}, {}, {}
    a = 0
    for b in oddE: oA2[b] = a; a += kbA[b]
    a = 0
    for b in oddE: oB2[b] = a; a += kbB[b]
    a = 0
    for b in oddN: oC2[b] = a; a += kbC[b]

    nc = bacc.Bacc("TRN2", num_devices=W)
    T = lambda n, s, d=bf16: nc.dram_tensor(n, s, d, kind="ExternalInput")
    x_blkT = T("x_blkT", [NBLK, 128, C])     # [b][cin%128][(k*2+q)*128+node]
    W1 = T("W1", [CONCAT, C_IN, C_HID])
    b1c = T("b1c", [1, C])
    W2p = T("W2p", [C, C_OUT_P])
    dv_blk = T("dv_blk", [128, NBLK], f32); dvsq_blk = T("dvsq_blk", [128, NBLK], f32)
    de_blk = T("de_blk", [128, EBLK], f32)
    idxA = T("idxA", [128, sumA * 8], i16); segA = T("segA", [128, sumA])
    idxB = T("idxB", [128, sumB * 8], i16); segB = T("segB", [128, sumB])
    idxC = T("idxC", [128, sumC * 8], i16); segC = T("segC", [128, sumC])
    slabD = T("slabD", [DSLAB, NCG, 128, 8, 128])
    slabE = T("slabE", [ESLAB, ECG, 128, 8, 128])
    iota_d = T("iota", [128, 128])
    out_own = nc.dram_tensor("out_own", [NPC, C_OUT_P], f32, kind="ExternalOutput")
    I = lambda n, s: nc.dram_tensor(n, s, bf16, kind="Internal")
    S = lambda n, s: nc.dram_tensor(n, s, bf16, kind="Internal", addr_space="Shared")
    y_own, y_full = I("y_own", [NPC, C]), S("y_full", [NP_, C])
    ef_own, ef_full = I("ef_own", [EPC, C]), S("ef_full", [EP_, C])
    y2_own, y2_full = I("y2_own", [NPC, C_OUT_P]), S("y2_full", [NP_, C_OUT_P])
    ef2_own, ef2_full = I("ef2_own", [EPC, C_OUT_P]), S("ef2_full", [EP_, C_OUT_P])
    RG = [list(range(W))]
    AG = lambda i, o: nc.gpsimd.collective_compute(
        "AllGather", mybir.AluOpType.bypass, replica_groups=RG, ins=[i[:]], outs=[o[:]])

    with TileContext(nc) as tc:
        with tc.tile_pool(name="const", bufs=1) as cp:
            w1_sb = cp.tile([128, CONCAT * 2 * C_HID], bf16)     # [k][q] -> 256 cols
            for k in range(CONCAT):
                for q in range(2):
                    nc.sync.dma_start(
                        w1_sb[:, (k * 2 + q) * C_HID:(k * 2 + q + 1) * C_HID],
                        W1[k, q * 128:(q + 1) * 128, :])
            w2_sb = cp.tile([128, 8 * C_OUT_P], bf16)
            for f in range(8):
                nc.sync.dma_start(w2_sb[:, f * C_OUT_P:(f + 1) * C_OUT_P],
                                  W2p[f * 128:(f + 1) * 128, :])
            b1_sb = cp.tile([1, C], bf16); nc.sync.dma_start(b1_sb[:], b1c[:])
            ones_sb = cp.tile([1, 128], bf16); nc.vector.memset(ones_sb[:], 1.0)
            iota_sb = cp.tile([128, 128], bf16); nc.sync.dma_start(iota_sb[:], iota_d[:])
            ident = cp.tile([128, 128], bf16); masks.make_identity(nc, ident[:])
            dv_sb = cp.tile([128, NBLK], f32); nc.sync.dma_start(dv_sb[:], dv_blk[:])
            dvsq_sb = cp.tile([128, NBLK], f32); nc.sync.dma_start(dvsq_sb[:], dvsq_blk[:])
            de_sb = cp.tile([128, EBLK], f32); nc.sync.dma_start(de_sb[:], de_blk[:])

            mm = lambda *a, **kw: nc.tensor.matmul(*a, skip_group_check=True, **kw)

            def seg_pass(kb, off, idx_sb, seg_sb, src_ap, elem, pool, ps,
                         start_stream, stop_stream):
                """Gather+one-hot-matmul accumulation for one block's stream."""
                for s in range(0, kb, BATCH):
                    nch = min(BATCH, kb - s)
                    k0 = off + s
                    g = pool.tile([128, BATCH, elem], bf16, tag="gat")
                    nc.gpsimd.dma_gather(
                        out_ap=g[:, :nch, :], in_ap=src_ap,
                        idxs_ap=idx_sb[:, k0 * 8:(k0 + nch) * 8],
                        num_idxs=nch * 128, num_idxs_reg=nch * 128,
                        elem_size=elem)
                    oh = pool.tile([128, BATCH, 128], bf16, tag="oh")
                    nc.vector.tensor_tensor(
                        out=oh[:, :nch, :],
                        in0=iota_sb[:, None, :].broadcast_to([128, nch, 128]),
                        in1=seg_sb[:, k0:k0 + nch, None].broadcast_to([128, nch, 128]),
                        op=mybir.AluOpType.is_equal)
                    for j in range(nch):
                        first = start_stream and (s == 0 and j == 0)
                        last = stop_stream and (s + j == kb - 1)
                        for h in range((elem + 511) // 512):
                            w_ = min(512, elem - h * 512)
                            mm(ps[:, h * 512:h * 512 + w_],
                               lhsT=oh[:, j, :], rhs=g[:, j, h * 512:h * 512 + w_],
                               start=first, stop=last)

            # ---- phase A: y = dv * (x @ W1 + 1 b1) ----
            with tc.tile_pool(name="pa", bufs=3) as pa, \
                 tc.tile_pool(name="pap", bufs=2, space="PSUM") as pap:
                for b in range(NBLK):
                    ps = pap.tile([128, C], f32, tag="psA")
                    mm(ps[:, :512], lhsT=ones_sb[:, :], rhs=b1_sb[:, :512], start=True, stop=False)
                    mm(ps[:, 512:], lhsT=ones_sb[:, :], rhs=b1_sb[:, 512:], start=True, stop=False)
                    xt = pa.tile([128, C], bf16, tag="xt")
                    nc.sync.dma_start(xt[:], x_blkT[b, :, :])
                    for k in range(CONCAT):
                        for q in range(2):
                            mm(ps[:, k * C_HID:(k + 1) * C_HID],
                               lhsT=xt[:, (k * 2 + q) * 128:(k * 2 + q + 1) * 128],
                               rhs=w1_sb[:, (k * 2 + q) * C_HID:(k * 2 + q + 1) * C_HID],
                               start=False, stop=(q == 1))
                    y_sb = pa.tile([128, C], bf16, tag="ysb")
                    nc.vector.tensor_tensor(
                        out=y_sb[:], in0=ps[:],
                        in1=dv_sb[:, b:b + 1].broadcast_to([128, C]),
                        op=mybir.AluOpType.mult)
                    nc.sync.dma_start(y_own[b * 128:(b + 1) * 128, :], y_sb[:])
            AG(y_own, y_full)

            # ---- phases B+C share the idx/seg stream pool ----
            with tc.tile_pool(name="idxp", bufs=1) as ip:
                iA = ip.tile([128, sumA * 8], i16); nc.sync.dma_start(iA[:], idxA[:])
                iB = ip.tile([128, sumB * 8], i16); nc.sync.dma_start(iB[:], idxB[:])
                iC = ip.tile([128, sumC * 8], i16); nc.sync.dma_start(iC[:], idxC[:])
                sA = ip.tile([128, sumA], bf16); nc.sync.dma_start(sA[:], segA[:])
                sB = ip.tile([128, sumB], bf16); nc.sync.dma_start(sB[:], segB[:])
                sC = ip.tile([128, sumC], bf16); nc.sync.dma_start(sC[:], segC[:])

                # ---- phase B: ef = de * (H^T y) over own edges ----
                with tc.tile_pool(name="pb", bufs=3) as pb, \
                     tc.tile_pool(name="pbp", bufs=3, space="PSUM") as pbp:
                    for b in range(EBLK):
                        ps = pbp.tile([128, C], f32, tag="psB")
                        seg_pass(kbA[b], oA[b], iA, sA, y_full[0:NHALF, :], C,
                                 pb, ps, True, False)
                        seg_pass(kbB[b], oB[b], iB, sB, y_full[NHALF:NP_, :], C,
                                 pb, ps, False, True)
                        ef_sb = pb.tile([128, C], bf16, tag="efsb")
                        nc.vector.tensor_tensor(
                            out=ef_sb[:], in0=ps[:],
                            in1=de_sb[:, b:b + 1].broadcast_to([128, C]),
                            op=mybir.AluOpType.mult)
                        nc.sync.dma_start(ef_own[b * 128:(b + 1) * 128, :], ef_sb[:])
                AG(ef_own, ef_full)

                # ---- phase C: u = relu(H ef); y2 = dv^2 * (u @ W2) ----
                with tc.tile_pool(name="pc", bufs=3) as pc, \
                     tc.tile_pool(name="pcp", bufs=3, space="PSUM") as pcp, \
                     tc.tile_pool(name="pct", bufs=1, space="PSUM") as pct:
                    for b in range(NBLK):
                        pz = pcp.tile([128, C], f32, tag="psC")
                        seg_pass(kbC[b], oC[b], iC, sC, ef_full[:], C, pc, pz,
                                 True, True)
                        u_sb = pc.tile([128, C], bf16, tag="usb")
                        nc.scalar.activation(out=u_sb[:], in_=pz[:],
                                             func=mybir.ActivationFunctionType.Relu)
                        pt = pct.tile([128, C], bf16, tag="ptC")
                        for f in range(8):
                            nc.tensor.transpose(pt[:, f * 128:(f + 1) * 128],
                                                u_sb[:, f * 128:(f + 1) * 128], ident[:])
                        ut_sb = pc.tile([128, C], bf16, tag="utsb")
                        nc.vector.tensor_copy(ut_sb[:], pt[:])
                        po = pct.tile([128, C_OUT_P], f32, tag="poC")
                        for f in range(8):
                            mm(po[:], lhsT=ut_sb[:, f * 128:(f + 1) * 128],
                               rhs=w2_sb[:, f * C_OUT_P:(f + 1) * C_OUT_P],
                               start=(f == 0), stop=(f == 7))
                        y2_sb = pc.tile([128, C_OUT_P], bf16, tag="y2sb")
                        nc.vector.tensor_tensor(
                            out=y2_sb[:], in0=po[:],
                            in1=dvsq_sb[:, b:b + 1].broadcast_to([128, C_OUT_P]),
                            op=mybir.AluOpType.mult)
                        nc.sync.dma_start(y2_own[b * 128:(b + 1) * 128, :], y2_sb[:])
            AG(y2_own, y2_full)

            # ---- phase D: ef2 = de * (H^T y2) ----
            # b%3==0 blocks: dense slab matmuls (TensorE); rest: gather (GpSimd)
            with tc.tile_pool(name="pdy", bufs=1) as pdy, \
                 tc.tile_pool(name="pdi", bufs=1) as pdi, \
                 tc.tile_pool(name="pd", bufs=3) as pd, \
                 tc.tile_pool(name="pdg", bufs=2) as pdg, \
                 tc.tile_pool(name="pdp", bufs=4, space="PSUM") as pdp:
                y2_sbuf = pdy.tile([128, NCH * C_OUT_P], bf16)
                for g in range(NCH):
                    nc.sync.dma_start(y2_sbuf[:, g * C_OUT_P:(g + 1) * C_OUT_P],
                                      y2_full[g * 128:(g + 1) * 128, :])
                iA2 = pdi.tile([128, sumA2 * 8], i16)
                iB2 = pdi.tile([128, sumB2 * 8], i16)
                sA2 = pdi.tile([128, sumA2], bf16)
                sB2 = pdi.tile([128, sumB2], bf16)
                for b in oddE:
                    nc.sync.dma_start(iA2[:, oA2[b] * 8:(oA2[b] + kbA[b]) * 8],
                                      idxA[:, oA[b] * 8:(oA[b] + kbA[b]) * 8])
                    nc.sync.dma_start(iB2[:, oB2[b] * 8:(oB2[b] + kbB[b]) * 8],
                                      idxB[:, oB[b] * 8:(oB[b] + kbB[b]) * 8])
                    nc.sync.dma_start(sA2[:, oA2[b]:oA2[b] + kbA[b]],
                                      segA[:, oA[b]:oA[b] + kbA[b]])
                    nc.sync.dma_start(sB2[:, oB2[b]:oB2[b] + kbB[b]],
                                      segB[:, oB[b]:oB[b] + kbB[b]])
                for b in range(EBLK):
                    ps2 = pdp.tile([128, C_OUT_P], f32, tag="psD")
                    if b % 3 == 0:
                        sb = b // 3
                        for g in range(NCG):
                            hD = pd.tile([128, 8, 128], bf16, tag="hD")
                            nc.sync.dma_start(hD[:], slabD[sb, g, :, :, :])
                            for i in range(8):
                                nc_ = g * 8 + i
                                mm(ps2[:], lhsT=hD[:, i, :],
                                   rhs=y2_sbuf[:, nc_ * C_OUT_P:(nc_ + 1) * C_OUT_P],
                                   start=(nc_ == 0), stop=(nc_ == NCH - 1))
                    else:
                        seg_pass(kbA[b], oA2[b], iA2, sA2, y2_full[0:NHALF, :],
                                 C_OUT_P, pdg, ps2, True, False)
                        seg_pass(kbB[b], oB2[b], iB2, sB2, y2_full[NHALF:NP_, :],
                                 C_OUT_P, pdg, ps2, False, True)
                    e2_sb = pd.tile([128, C_OUT_P], bf16, tag="e2sb")
                    nc.vector.tensor_tensor(
                        out=e2_sb[:], in0=ps2[:],
                        in1=de_sb[:, b:b + 1].broadcast_to([128, C_OUT_P]),
                        op=mybir.AluOpType.mult)
                    nc.sync.dma_start(ef2_own[b * 128:(b + 1) * 128, :], e2_sb[:])
            AG(ef2_own, ef2_full)

            # ---- phase E: res = dv * (H ef2) ----
            with tc.tile_pool(name="pey", bufs=1) as pey, \
                 tc.tile_pool(name="pei", bufs=1) as pei, \
                 tc.tile_pool(name="pe", bufs=3) as pe_, \
                 tc.tile_pool(name="peg", bufs=2) as peg, \
                 tc.tile_pool(name="pep", bufs=4, space="PSUM") as pep:
                ef2_sbuf = pey.tile([128, ECH * C_OUT_P], bf16)
                for g in range(ECH):
                    nc.sync.dma_start(ef2_sbuf[:, g * C_OUT_P:(g + 1) * C_OUT_P],
                                      ef2_full[g * 128:(g + 1) * 128, :])
                iC2 = pei.tile([128, sumC2 * 8], i16)
                sC2 = pei.tile([128, sumC2], bf16)
                for b in oddN:
                    nc.sync.dma_start(iC2[:, oC2[b] * 8:(oC2[b] + kbC[b]) * 8],
                                      idxC[:, oC[b] * 8:(oC[b] + kbC[b]) * 8])
                    nc.sync.dma_start(sC2[:, oC2[b]:oC2[b] + kbC[b]],
                                      segC[:, oC[b]:oC[b] + kbC[b]])
                for b in range(NBLK):
                    pz2 = pep.tile([128, C_OUT_P], f32, tag="psE")
                    if b % 3 == 0:
                        sb = b // 3
                        for g in range(ECG):
                            hE = pe_.tile([128, 8, 128], bf16, tag="hE")
                            nc.sync.dma_start(hE[:], slabE[sb, g, :, :, :])
                            for i in range(8):
                                ec_ = g * 8 + i
                                mm(pz2[:], lhsT=hE[:, i, :],
                                   rhs=ef2_sbuf[:, ec_ * C_OUT_P:(ec_ + 1) * C_OUT_P],
                                   start=(ec_ == 0), stop=(ec_ == ECH - 1))
                    else:
                        seg_pass(kbC[b], oC2[b], iC2, sC2, ef2_full[:], C_OUT_P,
                                 peg, pz2, True, True)
                    o_sb = pe_.tile([128, C_OUT_P], f32, tag="osb")
                    nc.vector.tensor_tensor(
                        out=o_sb[:], in0=pz2[:],
                        in1=dv_sb[:, b:b + 1].broadcast_to([128, C_OUT_P]),
                        op=mybir.AluOpType.mult)
                    nc.sync.dma_start(out_own[b * 128:(b + 1) * 128, :], o_sb[:])
    nc.finalize()
    return nc


_CACHE = {}


def kernel(x_list, W1, b1, W2, b2, node_idx, edge_idx, n_edges, _trace=False,
           _tmpdir=None):
    from concourse import bass_utils
    x_list = np.asarray(x_list, np.float32); W1 = np.asarray(W1, np.float32)
    b1 = np.asarray(b1, np.float32); W2 = np.asarray(W2, np.float32)
    b2 = np.asarray(b2, np.float32)
    node_idx = np.asarray(node_idx, np.int32); edge_idx = np.asarray(edge_idx, np.int32)

    dv = np.bincount(node_idx, minlength=N).astype(np.float32)
    de = np.bincount(edge_idx, minlength=E).astype(np.float32)
    dv_is = np.where(dv > 0, 1.0 / np.sqrt(np.maximum(dv, 1.0)), 0.0).astype(np.float32)
    de_inv = np.where(de > 0, 1.0 / np.maximum(de, 1.0), 0.0).astype(np.float32)
    # s1 = S @ 1 for the host-side b2 rank-1 term
    ef_t = np.bincount(edge_idx, weights=dv_is[node_idx], minlength=E) * de_inv
    s1 = dv_is * np.bincount(node_idx, weights=ef_t[edge_idx], minlength=N)

    cores, kbA, kbB, kbC = _prep(node_idx, edge_idx, dv_is, de_inv)
    key = (kbA, kbB, kbC)
    if key not in _CACHE:
        _CACHE[key] = _build(kbA, kbB, kbC)
    nc = _CACHE[key]

    W2p = np.zeros((C, C_OUT_P), np.float32)
    W2p[:, :C_OUT] = W2
    iota_np = np.tile(np.arange(128, dtype=np.float32), (128, 1))
    in_maps = []
    for c in range(W):
        # x_blkT[b, p, (k*2+q)*128+j] = x[k, node c*NPC_R + b*128+j, q*128+p]
        xc = np.zeros((CONCAT, NPC, C_IN), np.float32)
        xc[:, :NPC_R, :] = x_list[:, c * NPC_R:(c + 1) * NPC_R, :]
        xb = xc.reshape(CONCAT, NBLK, 128, 2, 128)      # k, b, j, q, p
        xb = xb.transpose(1, 4, 0, 3, 2).reshape(NBLK, 128, C)  # b, p, (k,q,j)
        m = dict(x_blkT=xb.astype(BF16), W1=W1.astype(BF16),
                 b1c=b1.reshape(1, C).astype(BF16), W2p=W2p.astype(BF16),
                 iota=iota_np.astype(BF16), **cores[c])
        in_maps.append(m)
    try:
        res = bass_utils.run_bass_kernel_spmd(nc, in_maps, core_ids=list(range(W)),
                                              trace=_trace, tmpdir=_tmpdir)
    except ModuleNotFoundError:
        res = bass_utils.run_bass_kernel_spmd(nc, in_maps, core_ids=list(range(W)),
                                              trace=False)
    out = np.empty((N, C_OUT), np.float32)
    for c in range(W):
        out[c * NPC_R:(c + 1) * NPC_R] = res.results[c]["out_own"][:NPC_R, :C_OUT]
    out += np.outer(s1, b2)
    kernel._last = res
    return out


# revision 14
# speedup vs baseline: 1.1215x; 1.1215x over previous
"""LAHGCN hypergraph-conv kernel for 8 Trainium2 NeuronCores (bf16).

Math (per reference):
  smooth(x) = Dv^-1/2 H De^-1 H^T Dv^-1/2 x  (S),  branches k=0..3:
  hidden_k = relu(S(x_k W1_k + 1 b1_k));  out = concat(hidden) W2 + b2;  res = S out.

Strategy: nodes sharded 8-way, edges sharded 8-way, AllGather between sides.
First smooth (C=1024): indirect dma_gather of rows + one-hot matmul on
TensorE with statically-baked index/segment streams (bf16 tables).
Second smooth (C=40, padded 128): hybrid - even blocks stream a host-built
dense-H block slab (bf16 counts) through plain TensorE matmuls (zero
descriptor-generation), odd blocks use dma_gather + one-hot (GpSimd); the
two routes run on different engines concurrently.
Degree scalings folded into y (dv), the edge passes (de) and post-W2 (dv^2);
b1 via rank-1 matmul; b2 via host-side rank-1 s1 = S@1 correction.
"""
import numpy as np
import ml_dtypes

BF16 = ml_dtypes.bfloat16
N, E, NNZ = 50000, 20000, 1600000
CONCAT, C_IN, C_HID = 4, 256, 256
C = CONCAT * C_HID            # 1024
C_OUT, C_OUT_P = 40, 128
W = 8
NPC_R, EPC_R = N // W, E // W           # 6250, 2500 real per core
NBLK, EBLK = 49, 20
NPC, EPC = NBLK * 128, EBLK * 128       # 6272, 2560 padded per core
NP_, EP_ = W * NPC, W * EPC             # 50176, 20480
NCH, ECH = NP_ // 128, EP_ // 128       # 392, 160 global 128-chunks
NCG, ECG = NCH // 8, ECH // 8           # 49, 20 groups of 8 chunks
DSLAB, ESLAB = (EBLK + 1) // 2, (NBLK + 1) // 2   # 10, 25 even blocks
NHALF = NP_ // 2                        # 25088
BATCH = 8                               # gather chunks per dma_gather (1024
                                        # idx = the max one gather supports)


def _wrap_idx(idx):
    """[L] int -> [128, L/16] int16 wrapped layout, replicated across q7 cores."""
    L = len(idx)
    assert L % 16 == 0
    a = np.full((16, L // 16), 0, np.int16)
    a[np.arange(L) % 16, np.arange(L) // 16] = idx.astype(np.int16)
    return np.tile(a, (8, 1))


def _streams_var(rows, segpos, kbs):
    """Flat index stream + seg table with per-block chunk counts kbs."""
    total = sum(kbs)
    idx = np.zeros(total * 128, np.int64)
    seg = np.full((128, total), -1.0, np.float32)
    off = 0
    for r, p, kb in zip(rows, segpos, kbs):
        n = len(r)
        assert n <= kb * 128
        idx[off * 128:off * 128 + n] = r
        cols = off + np.arange(n) // 128
        seg[np.arange(n) % 128, cols] = p.astype(np.float32)
        off += kb
    return idx, seg.astype(BF16)


def _prep(node_idx, edge_idx, dv_is, de_inv):
    """Host-side prep: gather streams for the first smooth + odd blocks of
    the second, dense block slabs for the second smooth's even blocks."""
    nrow = (node_idx // NPC_R) * NPC + node_idx % NPC_R    # node -> y row
    erow = (edge_idx // EPC_R) * EPC + edge_idx % EPC_R    # edge -> ef row
    p1 = np.argsort(edge_idx, kind="stable")
    e1, n1 = edge_idx[p1], nrow[p1]
    p2 = np.argsort(node_idx, kind="stable")
    n2, e2 = node_idx[p2], erow[p2]
    per = []
    for c in range(W):
        m1 = (e1 >= c * EPC_R) & (e1 < (c + 1) * EPC_R)
        el = e1[m1] - c * EPC_R
        nr = n1[m1]
        lo_rows, lo_pos, hi_rows, hi_pos = [], [], [], []
        for b in range(EBLK):
            mb = (el >= b * 128) & (el < (b + 1) * 128)
            rb, pb = nr[mb], el[mb] - b * 128
            lo = rb < NHALF
            lo_rows.append(rb[lo]); lo_pos.append(pb[lo])
            hi_rows.append(rb[~lo] - NHALF); hi_pos.append(pb[~lo])
        m2 = (n2 >= c * NPC_R) & (n2 < (c + 1) * NPC_R)
        nl = n2[m2] - c * NPC_R
        er = e2[m2]
        c_rows, c_pos = [], []
        for b in range(NBLK):
            mb = (nl >= b * 128) & (nl < (b + 1) * 128)
            c_rows.append(er[mb]); c_pos.append(nl[mb] - b * 128)
        per.append((lo_rows, lo_pos, hi_rows, hi_pos, c_rows, c_pos,
                    el, nr, nl, er))
    kbA = [max(1, max((len(p[0][b]) + 127) // 128 for p in per)) for b in range(EBLK)]
    kbB = [max(1, max((len(p[2][b]) + 127) // 128 for p in per)) for b in range(EBLK)]
    kbC = [max(1, max((len(p[4][b]) + 127) // 128 for p in per)) for b in range(NBLK)]
    cores = []
    for c in range(W):
        lo_rows, lo_pos, hi_rows, hi_pos, c_rows, c_pos, el, nr, nl, er = per[c]
        iA, sA = _streams_var(lo_rows, lo_pos, kbA)
        iB, sB = _streams_var(hi_rows, hi_pos, kbB)
        iC, sC = _streams_var(c_rows, c_pos, kbC)
        dv = np.zeros(NPC, np.float32)
        dv[:NPC_R] = dv_is[c * NPC_R:(c + 1) * NPC_R]
        de = np.zeros(EPC, np.float32)
        de[:EPC_R] = de_inv[c * EPC_R:(c + 1) * EPC_R]
        # dense slabD[n_pad, e_loc] counts, even edge blocks only
        sD = np.zeros((NP_, EPC), np.float32)
        np.add.at(sD, (nr, el), 1.0)
        sD = sD.reshape(NCH, 128, EBLK, 128).transpose(2, 0, 1, 3)[0::2]
        sD = sD.reshape(DSLAB, NCG, 8, 128, 128).transpose(0, 1, 3, 2, 4)
        slabD = sD.astype(BF16)
        del sD
        # dense slabE[e_pad, n_loc] counts, even node blocks only
        sE = np.zeros((EP_, NPC), np.float32)
        np.add.at(sE, (er, nl), 1.0)
        sE = sE.reshape(ECH, 128, NBLK, 128).transpose(2, 0, 1, 3)[0::2]
        sE = sE.reshape(ESLAB, ECG, 8, 128, 128).transpose(0, 1, 3, 2, 4)
        slabE = sE.astype(BF16)
        del sE
        cores.append(dict(
            idxA=_wrap_idx(iA), segA=sA, idxB=_wrap_idx(iB), segB=sB,
            idxC=_wrap_idx(iC), segC=sC, slabD=slabD, slabE=slabE,
            dv_blk=dv.reshape(NBLK, 128).T.copy(),
            dvsq_blk=(dv * dv).reshape(NBLK, 128).T.copy(),
            de_blk=de.reshape(EBLK, 128).T.copy()))
    return cores, tuple(kbA), tuple(kbB), tuple(kbC)


def _build(kbA, kbB, kbC):
    import concourse.bass as bass
    import concourse.mybir as mybir
    from concourse import bacc, masks
    from concourse.tile import TileContext

    f32, bf16, i16 = mybir.dt.float32, mybir.dt.bfloat16, mybir.dt.int16
    sumA, sumB, sumC = sum(kbA), sum(kbB), sum(kbC)
    oA = np.concatenate([[0], np.cumsum(kbA)]).tolist()
    oB = np.concatenate([[0], np.cumsum(kbB)]).tolist()
    oC = np.concatenate([[0], np.cumsum(kbC)]).tolist()
    # packed offsets for the odd-block re-loads in phases D/E
    oddE = [b for b in range(EBLK) if b % 2 == 1]
    oddN = [b for b in range(NBLK) if b % 2 == 1]
    sumA2 = sum(kbA[b] for b in oddE); sumB2 = sum(kbB[b] for b in oddE)
    sumC2 = sum(kbC[b] for b in oddN)
    oA2, oB2, oC2 = {}, {}, {}
    a = 0
    for b in oddE: oA2[b] = a; a += kbA[b]
    a = 0
    for b in oddE: oB2[b] = a; a += kbB[b]
    a = 0
    for b in oddN: oC2[b] = a; a += kbC[b]

    nc = bacc.Bacc("TRN2", num_devices=W)
    T = lambda n, s, d=bf16: nc.dram_tensor(n, s, d, kind="ExternalInput")
    x_blkT = T("x_blkT", [NBLK, 128, C])     # [b][cin%128][(k*2+q)*128+node]
    W1 = T("W1", [CONCAT, C_IN, C_HID])
    b1c = T("b1c", [1, C])
    W2p = T("W2p", [C, C_OUT_P])
    dv_blk = T("dv_blk", [128, NBLK], f32); dvsq_blk = T("dvsq_blk", [128, NBLK], f32)
    de_blk = T("de_blk", [128, EBLK], f32)
    idxA = T("idxA", [128, sumA * 8], i16); segA = T("segA", [128, sumA])
    idxB = T("idxB", [128, sumB * 8], i16); segB = T("segB", [128, sumB])
    idxC = T("idxC", [128, sumC * 8], i16); segC = T("segC", [128, sumC])
    slabD = T("slabD", [DSLAB, NCG, 128, 8, 128])
    slabE = T("slabE", [ESLAB, ECG, 128, 8, 128])
    iota_d = T("iota", [128, 128])
    out_own = nc.dram_tensor("out_own", [NPC, C_OUT_P], f32, kind="ExternalOutput")
    I = lambda n, s: nc.dram_tensor(n, s, bf16, kind="Internal")
    S = lambda n, s: nc.dram_tensor(n, s, bf16, kind="Internal", addr_space="Shared")
    y_own, y_full = I("y_own", [NPC, C]), S("y_full", [NP_, C])
    ef_own, ef_full = I("ef_own", [EPC, C]), S("ef_full", [EP_, C])
    y2_own, y2_full = I("y2_own", [NPC, C_OUT_P]), S("y2_full", [NP_, C_OUT_P])
    ef2_own, ef2_full = I("ef2_own", [EPC, C_OUT_P]), S("ef2_full", [EP_, C_OUT_P])
    RG = [list(range(W))]
    AG = lambda i, o: nc.gpsimd.collective_compute(
        "AllGather", mybir.AluOpType.bypass, replica_groups=RG, ins=[i[:]], outs=[o[:]])

    with TileContext(nc) as tc:
        with tc.tile_pool(name="const", bufs=1) as cp:
            w1_sb = cp.tile([128, CONCAT * 2 * C_HID], bf16)     # [k][q] -> 256 cols
            for k in range(CONCAT):
                for q in range(2):
                    nc.sync.dma_start(
                        w1_sb[:, (k * 2 + q) * C_HID:(k * 2 + q + 1) * C_HID],
                        W1[k, q * 128:(q + 1) * 128, :])
            w2_sb = cp.tile([128, 8 * C_OUT_P], bf16)
            for f in range(8):
                nc.sync.dma_start(w2_sb[:, f * C_OUT_P:(f + 1) * C_OUT_P],
                                  W2p[f * 128:(f + 1) * 128, :])
            b1_sb = cp.tile([1, C], bf16); nc.sync.dma_start(b1_sb[:], b1c[:])
            ones_sb = cp.tile([1, 128], bf16); nc.vector.memset(ones_sb[:], 1.0)
            iota_sb = cp.tile([128, 128], bf16); nc.sync.dma_start(iota_sb[:], iota_d[:])
            ident = cp.tile([128, 128], bf16); masks.make_identity(nc, ident[:])
            dv_sb = cp.tile([128, NBLK], f32); nc.sync.dma_start(dv_sb[:], dv_blk[:])
            dvsq_sb = cp.tile([128, NBLK], f32); nc.sync.dma_start(dvsq_sb[:], dvsq_blk[:])
            de_sb = cp.tile([128, EBLK], f32); nc.sync.dma_start(de_sb[:], de_blk[:])

            mm = lambda *a, **kw: nc.tensor.matmul(*a, skip_group_check=True, **kw)

            def seg_pass(kb, off, idx_sb, seg_sb, src_ap, elem, pool, ps,
                         start_stream, stop_stream):
                """Gather+one-hot-matmul accumulation for one block's stream."""
                for s in range(0, kb, BATCH):
                    nch = min(BATCH, kb - s)
                    k0 = off + s
                    g = pool.tile([128, BATCH, elem], bf16, tag="gat")
                    nc.gpsimd.dma_gather(
                        out_ap=g[:, :nch, :], in_ap=src_ap,
                        idxs_ap=idx_sb[:, k0 * 8:(k0 + nch) * 8],
                        num_idxs=nch * 128, num_idxs_reg=nch * 128,
                        elem_size=elem)
                    oh = pool.tile([128, BATCH, 128], bf16, tag="oh")
                    nc.vector.tensor_tensor(
                        out=oh[:, :nch, :],
                        in0=iota_sb[:, None, :].broadcast_to([128, nch, 128]),
                        in1=seg_sb[:, k0:k0 + nch, None].broadcast_to([128, nch, 128]),
                        op=mybir.AluOpType.is_equal)
                    for j in range(nch):
                        first = start_stream and (s == 0 and j == 0)
                        last = stop_stream and (s + j == kb - 1)
                        for h in range((elem + 511) // 512):
                            w_ = min(512, elem - h * 512)
                            mm(ps[:, h * 512:h * 512 + w_],
                               lhsT=oh[:, j, :], rhs=g[:, j, h * 512:h * 512 + w_],
                               start=first, stop=last)

            # ---- phase A: y = dv * (x @ W1 + 1 b1) ----
            with tc.tile_pool(name="pa", bufs=3) as pa, \
                 tc.tile_pool(name="pap", bufs=2, space="PSUM") as pap:
                for b in range(NBLK):
                    ps = pap.tile([128, C], f32, tag="psA")
                    mm(ps[:, :512], lhsT=ones_sb[:, :], rhs=b1_sb[:, :512], start=True, stop=False)
                    mm(ps[:, 512:], lhsT=ones_sb[:, :], rhs=b1_sb[:, 512:], start=True, stop=False)
                    xt = pa.tile([128, C], bf16, tag="xt")
                    nc.sync.dma_start(xt[:], x_blkT[b, :, :])
                    for k in range(CONCAT):
                        for q in range(2):
                            mm(ps[:, k * C_HID:(k + 1) * C_HID],
                               lhsT=xt[:, (k * 2 + q) * 128:(k * 2 + q + 1) * 128],
                               rhs=w1_sb[:, (k * 2 + q) * C_HID:(k * 2 + q + 1) * C_HID],
                               start=False, stop=(q == 1))
                    y_sb = pa.tile([128, C], bf16, tag="ysb")
                    nc.vector.tensor_tensor(
                        out=y_sb[:], in0=ps[:],
                        in1=dv_sb[:, b:b + 1].broadcast_to([128, C]),
                        op=mybir.AluOpType.mult)
                    nc.sync.dma_start(y_own[b * 128:(b + 1) * 128, :], y_sb[:])
            AG(y_own, y_full)

            # ---- phases B+C share the idx/seg stream pool ----
            with tc.tile_pool(name="idxp", bufs=1) as ip:
                iA = ip.tile([128, sumA * 8], i16); nc.sync.dma_start(iA[:], idxA[:])
                iB = ip.tile([128, sumB * 8], i16); nc.sync.dma_start(iB[:], idxB[:])
                iC = ip.tile([128, sumC * 8], i16); nc.sync.dma_start(iC[:], idxC[:])
                sA = ip.tile([128, sumA], bf16); nc.sync.dma_start(sA[:], segA[:])
                sB = ip.tile([128, sumB], bf16); nc.sync.dma_start(sB[:], segB[:])
                sC = ip.tile([128, sumC], bf16); nc.sync.dma_start(sC[:], segC[:])

                # ---- phase B: ef = de * (H^T y) over own edges ----
                with tc.tile_pool(name="pb", bufs=3) as pb, \
                     tc.tile_pool(name="pbp", bufs=3, space="PSUM") as pbp:
                    for b in range(EBLK):
                        ps = pbp.tile([128, C], f32, tag="psB")
                        seg_pass(kbA[b], oA[b], iA, sA, y_full[0:NHALF, :], C,
                                 pb, ps, True, False)
                        seg_pass(kbB[b], oB[b], iB, sB, y_full[NHALF:NP_, :], C,
                                 pb, ps, False, True)
                        ef_sb = pb.tile([128, C], bf16, tag="efsb")
                        nc.vector.tensor_tensor(
                            out=ef_sb[:], in0=ps[:],
                            in1=de_sb[:, b:b + 1].broadcast_to([128, C]),
                            op=mybir.AluOpType.mult)
                        nc.sync.dma_start(ef_own[b * 128:(b + 1) * 128, :], ef_sb[:])
                AG(ef_own, ef_full)

                # ---- phase C: u = relu(H ef); y2 = dv^2 * (u @ W2) ----
                with tc.tile_pool(name="pc", bufs=3) as pc, \
                     tc.tile_pool(name="pcp", bufs=3, space="PSUM") as pcp, \
                     tc.tile_pool(name="pct", bufs=1, space="PSUM") as pct:
                    for b in range(NBLK):
                        pz = pcp.tile([128, C], f32, tag="psC")
                        seg_pass(kbC[b], oC[b], iC, sC, ef_full[:], C, pc, pz,
                                 True, True)
                        u_sb = pc.tile([128, C], bf16, tag="usb")
                        nc.scalar.activation(out=u_sb[:], in_=pz[:],
                                             func=mybir.ActivationFunctionType.Relu)
                        pt = pct.tile([128, C], bf16, tag="ptC")
                        for f in range(8):
                            nc.tensor.transpose(pt[:, f * 128:(f + 1) * 128],
                                                u_sb[:, f * 128:(f + 1) * 128], ident[:])
                        ut_sb = pc.tile([128, C], bf16, tag="utsb")
                        nc.vector.tensor_copy(ut_sb[:], pt[:])
                        po = pct.tile([128, C_OUT_P], f32, tag="poC")
                        for f in range(8):
                            mm(po[:], lhsT=ut_sb[:, f * 128:(f + 1) * 128],
                               rhs=w2_sb[:, f * C_OUT_P:(f + 1) * C_OUT_P],
                               start=(f == 0), stop=(f == 7))
                        y2_sb = pc.tile([128, C_OUT_P], bf16, tag="y2sb")
                        nc.vector.tensor_tensor(
                            out=y2_sb[:], in0=po[:],
                            in1=dvsq_sb[:, b:b + 1].broadcast_to([128, C_OUT_P]),
                            op=mybir.AluOpType.mult)
                        nc.sync.dma_start(y2_own[b * 128:(b + 1) * 128, :], y2_sb[:])
            AG(y2_own, y2_full)

            # ---- phase D: ef2 = de * (H^T y2) ----
            # even blocks: dense slab matmuls (TensorE); odd: gather (GpSimd)
            with tc.tile_pool(name="pdy", bufs=1) as pdy, \
                 tc.tile_pool(name="pdi", bufs=1) as pdi, \
                 tc.tile_pool(name="pd", bufs=5) as pd, \
                 tc.tile_pool(name="pdg", bufs=2) as pdg, \
                 tc.tile_pool(name="pdp", bufs=4, space="PSUM") as pdp:
                y2_sbuf = pdy.tile([128, NCH * C_OUT_P], bf16)
                for g in range(NCH):
                    eng = nc.sync if g % 2 == 0 else nc.scalar
                    eng.dma_start(y2_sbuf[:, g * C_OUT_P:(g + 1) * C_OUT_P],
                                  y2_full[g * 128:(g + 1) * 128, :])
                iA2 = pdi.tile([128, sumA2 * 8], i16)
                iB2 = pdi.tile([128, sumB2 * 8], i16)
                sA2 = pdi.tile([128, sumA2], bf16)
                sB2 = pdi.tile([128, sumB2], bf16)
                for b in oddE:
                    nc.sync.dma_start(iA2[:, oA2[b] * 8:(oA2[b] + kbA[b]) * 8],
                                      idxA[:, oA[b] * 8:(oA[b] + kbA[b]) * 8])
                    nc.sync.dma_start(iB2[:, oB2[b] * 8:(oB2[b] + kbB[b]) * 8],
                                      idxB[:, oB[b] * 8:(oB[b] + kbB[b]) * 8])
                    nc.sync.dma_start(sA2[:, oA2[b]:oA2[b] + kbA[b]],
                                      segA[:, oA[b]:oA[b] + kbA[b]])
                    nc.sync.dma_start(sB2[:, oB2[b]:oB2[b] + kbB[b]],
                                      segB[:, oB[b]:oB[b] + kbB[b]])
                for b in range(EBLK):
                    ps2 = pdp.tile([128, C_OUT_P], f32, tag="psD")
                    if b % 2 == 0:
                        sb = b // 2
                        for g in range(NCG):
                            hD = pd.tile([128, 8, 128], bf16, tag="hD")
                            heng = nc.sync if g % 2 == 0 else nc.scalar
                            heng.dma_start(hD[:], slabD[sb, g, :, :, :])
                            for i in range(8):
                                nc_ = g * 8 + i
                                mm(ps2[:], lhsT=hD[:, i, :],
                                   rhs=y2_sbuf[:, nc_ * C_OUT_P:(nc_ + 1) * C_OUT_P],
                                   start=(nc_ == 0), stop=(nc_ == NCH - 1))
                    else:
                        seg_pass(kbA[b], oA2[b], iA2, sA2, y2_full[0:NHALF, :],
                                 C_OUT_P, pdg, ps2, True, False)
                        seg_pass(kbB[b], oB2[b], iB2, sB2, y2_full[NHALF:NP_, :],
                                 C_OUT_P, pdg, ps2, False, True)
                    e2_sb = pd.tile([128, C_OUT_P], bf16, tag="e2sb")
                    nc.vector.tensor_tensor(
                        out=e2_sb[:], in0=ps2[:],
                        in1=de_sb[:, b:b + 1].broadcast_to([128, C_OUT_P]),
                        op=mybir.AluOpType.mult)
                    nc.sync.dma_start(ef2_own[b * 128:(b + 1) * 128, :], e2_sb[:])
            AG(ef2_own, ef2_full)

            # ---- phase E: res = dv * (H ef2) ----
            with tc.tile_pool(name="pey", bufs=1) as pey, \
                 tc.tile_pool(name="pei", bufs=1) as pei, \
                 tc.tile_pool(name="pe", bufs=5) as pe_, \
                 tc.tile_pool(name="peg", bufs=2) as peg, \
                 tc.tile_pool(name="pep", bufs=4, space="PSUM") as pep:
                ef2_sbuf = pey.tile([128, ECH * C_OUT_P], bf16)
                for g in range(ECH):
                    eng = nc.sync if g % 2 == 0 else nc.scalar
                    eng.dma_start(ef2_sbuf[:, g * C_OUT_P:(g + 1) * C_OUT_P],
                                  ef2_full[g * 128:(g + 1) * 128, :])
                iC2 = pei.tile([128, sumC2 * 8], i16)
                sC2 = pei.tile([128, sumC2], bf16)
                for b in oddN:
                    nc.sync.dma_start(iC2[:, oC2[b] * 8:(oC2[b] + kbC[b]) * 8],
                                      idxC[:, oC[b] * 8:(oC[b] + kbC[b]) * 8])
                    nc.sync.dma_start(sC2[:, oC2[b]:oC2[b] + kbC[b]],
                                      segC[:, oC[b]:oC[b] + kbC[b]])
                for b in range(NBLK):
                    pz2 = pep.tile([128, C_OUT_P], f32, tag="psE")
                    if b % 2 == 0:
                        sb = b // 2
                        for g in range(ECG):
                            hE = pe_.tile([128, 8, 128], bf16, tag="hE")
                            heng = nc.sync if g % 2 == 0 else nc.scalar
                            heng.dma_start(hE[:], slabE[sb, g, :, :, :])
                            for i in range(8):
                                ec_ = g * 8 + i
                                mm(pz2[:], lhsT=hE[:, i, :],
                                   rhs=ef2_sbuf[:, ec_ * C_OUT_P:(ec_ + 1) * C_OUT_P],
                                   start=(ec_ == 0), stop=(ec_ == ECH - 1))
                    else:
                        seg_pass(kbC[b], oC2[b], iC2, sC2, ef2_full[:], C_OUT_P,
                                 peg, pz2, True, True)
                    o_sb = pe_.tile([128, C_OUT_P], f32, tag="osb")
                    nc.vector.tensor_tensor(
                        out=o_sb[:], in0=pz2[:],
                        in1=dv_sb[:, b:b + 1].broadcast_to([128, C_OUT_P]),
                        op=mybir.AluOpType.mult)
                    nc.sync.dma_start(out_own[b * 128:(b + 1) * 128, :], o_sb[:])
    nc.finalize()
    return nc


_CACHE = {}


def kernel(x_list, W1, b1, W2, b2, node_idx, edge_idx, n_edges, _trace=False,
           _tmpdir=None):
    from concourse import bass_utils
    x_list = np.asarray(x_list, np.float32); W1 = np.asarray(W1, np.float32)
    b1 = np.asarray(b1, np.float32); W2 = np.asarray(W2, np.float32)
    b2 = np.asarray(b2, np.float32)
    node_idx = np.asarray(node_idx, np.int32); edge_idx = np.asarray(edge_idx, np.int32)

    dv = np.bincount(node_idx, minlength=N).astype(np.float32)
    de = np.bincount(edge_idx, minlength=E).astype(np.float32)
    dv_is = np.where(dv > 0, 1.0 / np.sqrt(np.maximum(dv, 1.0)), 0.0).astype(np.float32)
    de_inv = np.where(de > 0, 1.0 / np.maximum(de, 1.0), 0.0).astype(np.float32)
    # s1 = S @ 1 for the host-side b2 rank-1 term
    ef_t = np.bincount(edge_idx, weights=dv_is[node_idx], minlength=E) * de_inv
    s1 = dv_is * np.bincount(node_idx, weights=ef_t[edge_idx], minlength=N)

    cores, kbA, kbB, kbC = _prep(node_idx, edge_idx, dv_is, de_inv)
    key = (kbA, kbB, kbC)
    if key not in _CACHE:
        _CACHE[key] = _build(kbA, kbB, kbC)
    nc = _CACHE[key]

    W2p = np.zeros((C, C_OUT_P), np.float32)
    W2p[:, :C_OUT] = W2
    iota_np = np.tile(np.arange(128, dtype=np.float32), (128, 1))
    in_maps = []
    for c in range(W):
        # x_blkT[b, p, (k*2+q)*128+j] = x[k, node c*NPC_R + b*128+j, q*128+p]
        xc = np.zeros((CONCAT, NPC, C_IN), np.float32)
        xc[:, :NPC_R, :] = x_list[:, c * NPC_R:(c + 1) * NPC_R, :]
        xb = xc.reshape(CONCAT, NBLK, 128, 2, 128)      # k, b, j, q, p
        xb = xb.transpose(1, 4, 0, 3, 2).reshape(NBLK, 128, C)  # b, p, (k,q,j)
        m = dict(x_blkT=xb.astype(BF16), W1=W1.astype(BF16),
                 b1c=b1.reshape(1, C).astype(BF16), W2p=W2p.astype(BF16),
                 iota=iota_np.astype(BF16), **cores[c])
        in_maps.append(m)
    try:
        res = bass_utils.run_bass_kernel_spmd(nc, in_maps, core_ids=list(range(W)),
                                              trace=_trace, tmpdir=_tmpdir)
    except ModuleNotFoundError:
        res = bass_utils.run_bass_kernel_spmd(nc, in_maps, core_ids=list(range(W)),
                                              trace=False)
    out = np.empty((N, C_OUT), np.float32)
    for c in range(W):
        out[c * NPC_R:(c + 1) * NPC_R] = res.results[c]["out_own"][:NPC_R, :C_OUT]
    out += np.outer(s1, b2)
    kernel._last = res
    return out
